# revision 1
# baseline (speedup 1.0000x reference)
"""Trainium2 Bass kernel for nn_ConvMultiHeadAttn.

Reference computation (per batch b):
  qkv = x @ Wqkv ; q,k1,k2,v = split(qkv)            [L, 4D]
  s1 = q @ k1^T ; s2 = q @ k2^T   (per head)         [H, L, L]
  attn = where(qmask_q == qmask_k, s1, s2)
  attn = where(mask_k, attn, -1e9) + dis             dis = -(shift*(tq-tk)^2 + bias_p)
  out = softmax(attn) @ v ; out = out @ Wfc + bfc

Strategy: data-parallel over batch (2 batches per NeuronCore, 8 cores, no
collectives). Scores are computed k-major (scores^T[k, q]) so the softmax
denominator and the attn@v contraction are matmuls.  The qmask select is a
single predicated copy on DVE; the mask bias rides the exp() per-partition
bias; the Gaussian bias is folded in as exp(dis) (host precomputed) via one
elementwise multiply on GpSimd.  Softmax needs no max pass (logits bounded,
exp stored in fp32->bf16) and the denominator comes free as a ones column in
the attn@v matmul.  fp16 operands on the q/k side (score accuracy), bf16 on
the v/fc side.
"""

import numpy as np
import ml_dtypes

import concourse.bass as bass
import concourse.bacc as bacc
import concourse.mybir as mybir
import concourse.tile as tile
from concourse.bass_utils import run_bass_kernel_spmd
from concourse.masks import make_identity

B, L, D, H = 16, 512, 1024, 16
DH = D // H            # 64
NCORES = 8
BPC = B // NCORES      # batches per core
KC = L // 128          # 4 token chunks
DCH = D // 128         # 8 d-model chunks
NEG = -1e9

F16 = mybir.dt.float16
BF16 = mybir.dt.bfloat16
F32 = mybir.dt.float32
EXP = mybir.ActivationFunctionType.Exp


def _build_bass():
    nc = bacc.Bacc(trn_type="TRN2")
    xb = nc.dram_tensor("xb", [BPC, L, D], F16, kind="ExternalInput")
    wqkv = nc.dram_tensor("wqkv", [D, 4 * D], F16, kind="ExternalInput")
    wfc = nc.dram_tensor("wfc", [D, D], BF16, kind="ExternalInput")
    bfc = nc.dram_tensor("bfc", [D], F32, kind="ExternalInput")
    same = nc.dram_tensor("same", [BPC, L, L], mybir.dt.uint8, kind="ExternalInput")
    gdis = nc.dram_tensor("gdis", [L, L], BF16, kind="ExternalInput")
    kbias = nc.dram_tensor("kbias", [128, BPC * KC], F32, kind="ExternalInput")
    outp = nc.dram_tensor("outp", [BPC, L, D], F32, kind="ExternalOutput")

    with tile.TileContext(nc) as tc:
        with (
            tc.tile_pool(name="w", bufs=1) as wpool,
            tc.tile_pool(name="a", bufs=1) as apool,
            tc.tile_pool(name="x", bufs=2) as xpool,
            tc.tile_pool(name="e", bufs=4) as epool,
            tc.tile_pool(name="d", bufs=2) as dpool,
            tc.tile_pool(name="o", bufs=4) as opool,
            tc.tile_pool(name="mm", bufs=2, space="PSUM") as mmps,
            tc.tile_pool(name="sc", bufs=4, space="PSUM") as scps,
            tc.tile_pool(name="av", bufs=2, space="PSUM") as avps,
        ):
            # resident weights / constants
            wq_sb = wpool.tile([128, DCH, 4 * D], F16)
            nc.sync.dma_start(
                out=wq_sb, in_=wqkv[:, :].rearrange("(c p) f -> p c f", p=128)
            )
            wf_sb = wpool.tile([128, DCH, D], BF16)
            nc.sync.dma_start(
                out=wf_sb, in_=wfc[:, :].rearrange("(c p) f -> p c f", p=128)
            )
            bf_sb = wpool.tile([128, D], F32)
            nc.sync.dma_start(out=bf_sb, in_=bfc[:].unsqueeze(0).broadcast_to([128, D]))
            kb_sb = wpool.tile([128, BPC * KC], F32)
            nc.sync.dma_start(out=kb_sb, in_=kbias[:, :])
            gd_sb = wpool.tile([128, KC, L], BF16)
            nc.sync.dma_start(
                out=gd_sb, in_=gdis[:, :].rearrange("(c p) q -> p c q", p=128)
            )
            ident = wpool.tile([128, 128], F16)
            make_identity(nc, ident)

            # round-robin PSUM->SBUF evacuation between DVE and ACT
            rr = [0]

            def evac(dst, src):
                if rr[0] % 2 == 0:
                    nc.vector.tensor_copy(out=dst, in_=src)
                else:
                    nc.scalar.copy(out=dst, in_=src)
                rr[0] += 1

            for b in range(BPC):
                sm = xpool.tile([128, KC, L], mybir.dt.uint8, tag="sm")
                nc.sync.dma_start(
                    out=sm, in_=same[b, :, :].rearrange("(c p) q -> p c q", p=128)
                )
                # x tok-major load, then x^T (d on partitions) via PE transpose
                xin = xpool.tile([128, KC, D], F16, tag="xin")
                nc.sync.dma_start(
                    out=xin, in_=xb[b, :, :].rearrange("(t p) d -> p t d", p=128)
                )
                xT = xpool.tile([128, DCH, L], F16, tag="xT", bufs=1)
                for c in range(DCH):
                    for t in range(KC):
                        tp = mmps.tile([128, 128], F16, tag="mm")
                        nc.tensor.transpose(
                            tp, xin[:, t, c * 128 : (c + 1) * 128], ident
                        )
                        evac(xT[:, c, t * 128 : (t + 1) * 128], tp)

                qt = apool.tile([128, DCH, L], F16, tag="qt")
                kt1 = apool.tile([128, DCH, L], F16, tag="kt1")
                kt2 = apool.tile([128, DCH, L], F16, tag="kt2")
                vsb = apool.tile([128, KC, H, DH + 2], BF16, tag="vsb")
                ctxT = apool.tile([128, DCH, L], BF16, tag="ctxT")
                nc.vector.memset(vsb[:, :, :, DH : DH + 1], 1.0)

                # qkv projections.  Q/K1/K2 feature-major (k-major for scores),
                # V token-major (natural) for the attn@v stationary operand.
                for which, foff in ((qt, 0), (kt1, D), (kt2, 2 * D)):
                    for j in range(8):
                        ps = mmps.tile([128, L], F32, tag="mm")
                        for c in range(DCH):
                            nc.tensor.matmul(
                                ps,
                                lhsT=wq_sb[:, c, foff + j * 128 : foff + (j + 1) * 128],
                                rhs=xT[:, c, :],
                                start=(c == 0),
                                stop=(c == DCH - 1),
                            )
                        evac(which[:, j, :], ps)
                for t in range(KC):
                    for vc in range(2):
                        ps = mmps.tile([128, L], F32, tag="mm")
                        for c in range(DCH):
                            nc.tensor.matmul(
                                ps,
                                lhsT=xT[:, c, t * 128 : (t + 1) * 128],
                                rhs=wq_sb[:, c, 3 * D + vc * 512 : 3 * D + (vc + 1) * 512],
                                start=(c == 0),
                                stop=(c == DCH - 1),
                            )
                        evac(
                            vsb[:, t, vc * 8 : (vc + 1) * 8, 0:DH],
                            ps[:, :].rearrange("p (h e) -> p h e", h=8),
                        )

                # attention, two heads (one feature chunk) at a time
                Dall = dpool.tile([16, L], F32, tag="Dall")
                for hp in range(8):
                    h0, h1 = 2 * hp, 2 * hp + 1
                    av_a = avps.tile([DH + 1, L], F32, tag="av")
                    av_b = avps.tile([DH + 1, L], F32, tag="av")
                    for kc in range(KC):
                        ks = slice(kc * 128, (kc + 1) * 128)
                        s1a = scps.tile([128, L], F32, tag="sc")
                        s1b = scps.tile([128, L], F32, tag="sc")
                        s2a = scps.tile([128, L], F32, tag="sc")
                        s2b = scps.tile([128, L], F32, tag="sc")
                        # scores^T[k, q]; even head on PE rows 0-63, odd on 64-127
                        nc.tensor.matmul(s1a, lhsT=kt1[0:64, hp, ks], rhs=qt[0:64, hp, :])
                        nc.tensor.matmul(s1b, lhsT=kt1[64:128, hp, ks], rhs=qt[64:128, hp, :])
                        nc.tensor.matmul(s2a, lhsT=kt2[0:64, hp, ks], rhs=qt[0:64, hp, :])
                        nc.tensor.matmul(s2b, lhsT=kt2[64:128, hp, ks], rhs=qt[64:128, hp, :])
                        # attn = where(same, s1, s2), in place in s2
                        nc.vector.copy_predicated(out=s2a, mask=sm[:, kc, :], data=s1a)
                        nc.vector.copy_predicated(out=s2b, mask=sm[:, kc, :], data=s1b)
                        # exp(attn + keymask bias); then * exp(dis) on gpsimd
                        kb = kb_sb[:, b * KC + kc : b * KC + kc + 1]
                        ea = epool.tile([128, L], BF16, tag="ea", bufs=3)
                        eb = epool.tile([128, L], BF16, tag="eb", bufs=3)
                        nc.scalar.activation(out=ea, in_=s2a, func=EXP, bias=kb, scale=1.0)
                        nc.scalar.activation(out=eb, in_=s2b, func=EXP, bias=kb, scale=1.0)
                        pa = epool.tile([128, L], BF16, tag="pa", bufs=3)
                        pb = epool.tile([128, L], BF16, tag="pb", bufs=3)
                        nc.gpsimd.tensor_mul(pa, ea, gd_sb[:, kc, :])
                        nc.gpsimd.tensor_mul(pb, eb, gd_sb[:, kc, :])
                        # attn @ v with a ones column -> softmax denominator row
                        nc.tensor.matmul(
                            av_a, lhsT=vsb[:, kc, h0, 0 : DH + 1], rhs=pa,
                            start=(kc == 0), stop=(kc == KC - 1),
                        )
                        nc.tensor.matmul(
                            av_b, lhsT=vsb[:, kc, h1, 0 : DH + 1], rhs=pb,
                            start=(kc == 0), stop=(kc == KC - 1),
                        )
                    # evacuate ctx^T (unnormalized); odd head shifts to rows 64-127
                    nc.scalar.copy(out=ctxT[0:64, hp, :], in_=av_a[0:DH, :])
                    cu = epool.tile([64, L], BF16, tag="cu")
                    nc.scalar.copy(out=cu, in_=av_b[0:DH, :])
                    nc.sync.dma_start(out=ctxT[64:128, hp, :], in_=cu)
                    # denominator rows -> Dall
                    da = dpool.tile([65, L], F32, tag="da")
                    db = dpool.tile([65, L], F32, tag="db")
                    nc.vector.tensor_copy(out=da[64:65, :], in_=av_a[DH : DH + 1, :])
                    nc.vector.tensor_copy(out=db[64:65, :], in_=av_b[DH : DH + 1, :])
                    nc.sync.dma_start(out=Dall[h0 : h0 + 1, :], in_=da[64:65, :])
                    nc.sync.dma_start(out=Dall[h1 : h1 + 1, :], in_=db[64:65, :])

                # softmax normalization: ctxT *= broadcast(1/denom)
                Rf = dpool.tile([16, L], F32, tag="Rf")
                nc.vector.reciprocal_approx_fast(out=Rf, in_=Dall)
                Rb16 = dpool.tile([16, L], BF16, tag="Rb16")
                nc.vector.tensor_copy(out=Rb16, in_=Rf)
                for hp in range(8):
                    rb = opool.tile([128, L], BF16, tag="rb")
                    nc.sync.dma_start(
                        out=rb,
                        in_=Rb16[2 * hp : 2 * hp + 2, :].unsqueeze(1).broadcast_to([2, 64, L]),
                    )
                    nc.vector.tensor_mul(ctxT[:, hp, :], ctxT[:, hp, :], rb)

                # fc + bias
                for t in range(KC):
                    for oc in range(2):
                        ps = mmps.tile([128, 512], F32, tag="mm")
                        for c in range(DCH):
                            nc.tensor.matmul(
                                ps,
                                lhsT=ctxT[:, c, t * 128 : (t + 1) * 128],
                                rhs=wf_sb[:, c, oc * 512 : (oc + 1) * 512],
                                start=(c == 0),
                                stop=(c == DCH - 1),
                            )
                        ob = opool.tile([128, 512], F32, tag="ob")
                        nc.vector.tensor_add(ob, ps, bf_sb[:, oc * 512 : (oc + 1) * 512])
                        nc.sync.dma_start(
                            out=outp[b, t * 128 : (t + 1) * 128, oc * 512 : (oc + 1) * 512],
                            in_=ob,
                        )
    return nc


_NC_CACHE = None


def _get_nc():
    global _NC_CACHE
    if _NC_CACHE is None:
        nc = _build_bass()
        nc.finalize()
        _NC_CACHE = nc
    return _NC_CACHE


def kernel(x, mask, qmask, Wqkv, Wfc, bfc, shift, bias_p, use_Gaussian):
    bf16 = ml_dtypes.bfloat16
    x = np.asarray(x, dtype=np.float32)
    mask = np.asarray(mask)
    qmask = np.asarray(qmask)
    wq16 = np.ascontiguousarray(np.asarray(Wqkv, dtype=np.float32).astype(np.float16))
    wf16 = np.ascontiguousarray(np.asarray(Wfc, dtype=np.float32).astype(bf16))
    bfc32 = np.ascontiguousarray(np.asarray(bfc, dtype=np.float32))
    shift_v = float(np.asarray(shift, dtype=np.float64).reshape(-1)[0])
    bias_v = float(np.asarray(bias_p, dtype=np.float64).reshape(-1)[0])
    ug = bool(np.asarray(use_Gaussian).reshape(-1)[0])

    x16 = x.astype(np.float16)
    same_b = (qmask[:, :, None] == qmask[:, None, :]).astype(np.uint8)  # [B, L, L]
    idx = np.arange(L, dtype=np.float64)
    if ug:
        dis = -(shift_v * (idx[:, None] - idx[None, :]) ** 2 + bias_v)
    else:
        dis = np.zeros((L, L), dtype=np.float64)
    gdis_b = np.exp(dis).astype(bf16)
    keyadd = np.where(mask != 0, 0.0, NEG).astype(np.float32)  # [B, L]
    kb_all = keyadd.reshape(B, KC, 128).transpose(2, 0, 1)  # [128, B, KC]

    nc = _get_nc()
    in_maps = []
    for core in range(NCORES):
        bs = slice(core * BPC, (core + 1) * BPC)
        in_maps.append(
            {
                "xb": np.ascontiguousarray(x16[bs]),
                "wqkv": wq16,
                "wfc": wf16,
                "bfc": bfc32,
                "same": np.ascontiguousarray(same_b[bs]),
                "gdis": gdis_b,
                "kbias": np.ascontiguousarray(
                    kb_all[:, bs, :].reshape(128, BPC * KC)
                ),
            }
        )
    res = run_bass_kernel_spmd(nc, in_maps, core_ids=list(range(NCORES)))
    kernel.last_perf = res
    out = np.concatenate([r["outp"] for r in res.results], axis=0)
    return np.ascontiguousarray(out.astype(np.float32))



# revision 13
# speedup vs baseline: 1.2205x; 1.2205x over previous
"""Trainium2 Bass kernel for nn_ConvMultiHeadAttn.

Reference computation (per batch b):
  qkv = x @ Wqkv ; q,k1,k2,v = split(qkv)            [L, 4D]
  s1 = q @ k1^T ; s2 = q @ k2^T   (per head)         [H, L, L]
  attn = where(qmask_q == qmask_k, s1, s2)
  attn = where(mask_k, attn, -1e9) + dis             dis = -(shift*(tq-tk)^2 + bias_p)
  out = softmax(attn) @ v ; out = out @ Wfc + bfc

Strategy: data-parallel over batch (2 batches per core, 8 cores, no
collectives).  The Gaussian positional bias makes attention local: a key at
distance d carries a weight factor exp(-shift*d^2), so for shift >= ~0.04
every weight with |q-k| > 64 underflows to zero in fp32 relative to the
in-band maximum (validated numerically: the 128-banded softmax output is
bit-identical to the full one).  Scores are therefore computed only on a
128-wide sliding k-window, organized as 5 k-chunks offset by -64 from the
q-tiles so each q-tile's window is exactly 2 chunks.

Per (head-pair hp, chunk c): 4 score matmuls (s1/s2 x even/odd head, K=64)
ordered so consecutive pairs hit disjoint PE row groups AND disjoint PSUM
banks (they run concurrently); scores^T is k-major [k, q], two heads packed
per PSUM bank at columns 0/256.  qmask select = one DVE copy_predicated;
exp on ACT (key-mask bias rides the per-partition bias port); the exp(dis)
Gaussian factor depends only on q-k, so it is ONE fixed Toeplitz [128, 512]
bf16 tile multiplied on GpSimd (SBUF-only engine).  attn@v accumulates per
128-col q-tile region in PSUM with a ones column appended to V giving the
softmax denominator for free; normalization commutes to an in-place bf16
multiply after a batched reciprocal; fc runs with Wfc resident.

The attn@v matmuls for head-pair hp are issued one head-pair late (between
the score matmuls of hp+1) so the PE never waits on the DVE->ACT->GpSimd
elementwise pipeline: the HAM clock gate drops the PE to 1.2 GHz after any
idle window, so the program is ordered Q(0) A(0) Q(1) F(0) A(1) F(1) with
back-to-back matmuls throughout.

Fallback: if the runtime shift is too small for the 128 window (or
use_Gaussian=0), the original full-attention kernel is built instead.
"""

import numpy as np
import ml_dtypes

import concourse.bass as bass
import concourse.bacc as bacc
import concourse.mybir as mybir
import concourse.tile as tile
from concourse.bass_utils import run_bass_kernel_spmd
from concourse.masks import make_identity

B, L, D, H = 16, 512, 1024, 16
DH = D // H            # 64
NCORES = 8
BPC = B // NCORES      # batches per core
KC = L // 128          # 4 token chunks
DCH = D // 128         # 8 d-model chunks
NCH = 5                # banded offset k-chunks
NEG = -1e9

F16 = mybir.dt.float16
BF16 = mybir.dt.bfloat16
F32 = mybir.dt.float32
U8 = mybir.dt.uint8
EXP = mybir.ActivationFunctionType.Exp

# Banded window validity: shift*65^2 must dominate logit spread + margin.
BAND_SHIFT_MIN = 0.0402

# chunk c covers tokens [128c-64, 128c+64) and serves q in [128(c-1), 128(c+1))
CHUNKS = []
for _c in range(NCH):
    _k0, _k1 = max(0, 128 * _c - 64), min(L, 128 * _c + 64)
    _q0, _q1 = max(0, 128 * (_c - 1)), min(L, 128 * (_c + 1))
    CHUNKS.append((_k0, _k1 - _k0, _q0, _q1 - _q0))
# av matmul schedule: q-tile t accumulates from chunks (t: start, t+1: stop)
AV_SCHED = []
for _t in range(4):
    AV_SCHED.append((_t, _t, True, False))
    AV_SCHED.append((_t + 1, _t, False, True))
AV_SCHED.sort(key=lambda x: x[0])


def _build_banded(use_bias_fc):
    nc = bacc.Bacc(trn_type="TRN2")
    xTd = nc.dram_tensor("xTd", [BPC, D, L], F16, kind="ExternalInput")
    wqkv = nc.dram_tensor("wqkv", [D, 4 * D], F16, kind="ExternalInput")
    wfc = nc.dram_tensor("wfc", [D, D], BF16, kind="ExternalInput")
    bfc = nc.dram_tensor("bfc", [D], F32, kind="ExternalInput")
    smd = nc.dram_tensor("smd", [BPC, 128, NCH, 512], U8, kind="ExternalInput")
    etabd = nc.dram_tensor("etabd", [128, 2, 512], BF16, kind="ExternalInput")
    kbias = nc.dram_tensor("kbias", [128, BPC * NCH], F32, kind="ExternalInput")
    outp = nc.dram_tensor("outp", [BPC, L, D], F16, kind="ExternalOutput")

    with tile.TileContext(nc) as tc:
        with (
            tc.tile_pool(name="w", bufs=1) as wpool,
            tc.tile_pool(name="a", bufs=2) as apool,
            tc.tile_pool(name="e", bufs=1) as epool,
            tc.tile_pool(name="o", bufs=1) as opool,
            tc.tile_pool(name="mm", bufs=2, space="PSUM") as mmps,
            tc.tile_pool(name="sc", bufs=2, space="PSUM") as scps,
            tc.tile_pool(name="av", bufs=1, space="PSUM") as avps,
        ):
            # ---- resident weights / constants ----
            wq_sb = wpool.tile([128, DCH, 4 * D], F16)
            for jg in range(8):  # chunked load so the first matmuls start early
                nc.sync.dma_start(
                    out=wq_sb[:, :, jg * 512 : (jg + 1) * 512],
                    in_=wqkv[:, jg * 512 : (jg + 1) * 512].rearrange(
                        "(c p) f -> p c f", p=128
                    ),
                )
            wf_sb = wpool.tile([128, DCH, D], BF16)
            nc.sync.dma_start(
                out=wf_sb, in_=wfc[:, :].rearrange("(c p) f -> p c f", p=128)
            )
            if use_bias_fc:
                bf_sb = wpool.tile([128, D], F32)
                nc.sync.dma_start(
                    out=bf_sb, in_=bfc[:].unsqueeze(0).broadcast_to([128, D])
                )
            etab = wpool.tile([128, 2, 512], BF16)
            nc.sync.dma_start(out=etab, in_=etabd[:, :, :])
            kb_sb = wpool.tile([128, BPC * NCH], F32)
            nc.sync.dma_start(out=kb_sb, in_=kbias[:, :])

            # round-robin PSUM->SBUF evacuation between DVE and ACT
            rr = [0]

            def evac(dst, src):
                if rr[0] % 2 == 0:
                    nc.vector.tensor_copy(out=dst, in_=src)
                else:
                    nc.scalar.copy(out=dst, in_=src)
                rr[0] += 1

            state = {}

            def phase_q(b):
                xT = apool.tile([128, DCH, L], F16, tag="xT", bufs=1)
                nc.sync.dma_start(
                    out=xT, in_=xTd[b, :, :].rearrange("(c p) t -> p c t", p=128)
                )
                sm = apool.tile([128, NCH, 512], U8, tag="sm", bufs=1)
                nc.scalar.dma_start(out=sm, in_=smd[b, :, :, :])

                # V projection first (token-major) so the offset-chunk
                # relayout DMAs complete while the q/k projections run.
                vtmp = apool.tile([128, KC, H, DH], BF16, tag="vtmp", bufs=1)
                for t in range(KC):
                    for vc in range(2):
                        ps = mmps.tile([128, 512], F32, tag="mm")
                        for c in range(DCH):
                            nc.tensor.matmul(
                                ps,
                                lhsT=xT[:, c, t * 128 : (t + 1) * 128],
                                rhs=wq_sb[
                                    :, c, 3 * D + vc * 512 : 3 * D + (vc + 1) * 512
                                ],
                                start=(c == 0),
                                stop=(c == DCH - 1),
                            )
                        evac(
                            vtmp[:, t, vc * 8 : (vc + 1) * 8, :],
                            ps[:, :].rearrange("p (h e) -> p h e", h=8),
                        )
                vsb = apool.tile([128, NCH, H, DH + 1], BF16, tag="vsb", bufs=1)
                nc.vector.memset(vsb[:, :, :, DH : DH + 1], 1.0)
                nc.scalar.dma_start(out=vsb[0:64, 0, :, 0:DH], in_=vtmp[0:64, 0, :, :])
                nc.scalar.dma_start(
                    out=vsb[0:64, 1:5, :, 0:DH], in_=vtmp[64:128, 0:4, :, :]
                )
                nc.scalar.dma_start(
                    out=vsb[64:128, 1:4, :, 0:DH], in_=vtmp[0:64, 1:4, :, :]
                )

                qt = apool.tile([128, DCH, L], F16, tag="qt")
                kt1 = apool.tile([128, DCH, L], F16, tag="kt1")
                kt2 = apool.tile([128, DCH, L], F16, tag="kt2")
                for which, foff in ((qt, 0), (kt1, D), (kt2, 2 * D)):
                    for j in range(8):
                        ps = mmps.tile([128, 512], F32, tag="mm")
                        for c in range(DCH):
                            nc.tensor.matmul(
                                ps,
                                lhsT=wq_sb[:, c, foff + j * 128 : foff + (j + 1) * 128],
                                rhs=xT[:, c, :],
                                start=(c == 0),
                                stop=(c == DCH - 1),
                            )
                        evac(which[:, j, :], ps)
                state[b] = (qt, kt1, kt2, vsb, sm)

            def scores_ew(b, hp, c, pa_tiles):
                qt, kt1, kt2, vsb, sm = state[b]
                k0, kn, q0, qn = CHUNKS[c]
                # one PSUM bank per matmul (concurrent row-tiled matmuls
                # must never share a bank: drains overlap -> fatal collision)
                s1a = scps.tile([128, 512], F32, tag="sc", bufs=4)
                s1b = scps.tile([128, 512], F32, tag="sc", bufs=4)
                s2a = scps.tile([128, 512], F32, tag="sc", bufs=4)
                s2b = scps.tile([128, 512], F32, tag="sc", bufs=4)
                ks = slice(k0, k0 + kn)
                qs = slice(q0, q0 + qn)
                # (h0 | h1) row-tiled pairs run concurrently, distinct banks
                nc.tensor.matmul(s1a[0:kn, 0:qn], lhsT=kt1[0:64, hp, ks], rhs=qt[0:64, hp, qs])
                nc.tensor.matmul(s1b[0:kn, 0:qn], lhsT=kt1[64:128, hp, ks], rhs=qt[64:128, hp, qs])
                nc.tensor.matmul(s2a[0:kn, 0:qn], lhsT=kt2[0:64, hp, ks], rhs=qt[0:64, hp, qs])
                nc.tensor.matmul(s2b[0:kn, 0:qn], lhsT=kt2[64:128, hp, ks], rhs=qt[64:128, hp, qs])

                ea = epool.tile([128, 512], BF16, tag="ea", bufs=3)
                pa = epool.tile([128, 512], BF16, tag="pa", bufs=9)
                kb = kb_sb[:, b * NCH + c : b * NCH + c + 1]
                ev = 0 if c == 0 else 1  # Toeplitz variant (edge chunk 0)
                for h, (sx1, sx2) in ((0, (s1a, s2a)), (1, (s1b, s2b))):
                    cs = slice(256 * h, 256 * h + qn)
                    nc.vector.copy_predicated(
                        out=sx2[0:kn, 0:qn], mask=sm[0:kn, c, 0:qn], data=sx1[0:kn, 0:qn]
                    )
                    nc.scalar.activation(
                        out=ea[0:kn, cs], in_=sx2[0:kn, 0:qn], func=EXP,
                        bias=kb[0:kn, :], scale=1.0,
                    )
                if qn == 256:
                    nc.gpsimd.tensor_mul(
                        pa[0:kn, :], ea[0:kn, :], etab[0:kn, ev, :]
                    )
                else:
                    for h in range(2):
                        cs = slice(256 * h, 256 * h + qn)
                        nc.gpsimd.tensor_mul(
                            pa[0:kn, cs], ea[0:kn, cs], etab[0:kn, ev, cs]
                        )
                pa_tiles[(hp, c)] = pa

            def phase_a(b):
                vsb = state[b][3]
                ctxT = apool.tile([128, DCH, L], BF16, tag="ctxT", bufs=1)
                zall = opool.tile([16, 512], BF16, tag="zall", bufs=1)
                pa_tiles = {}
                for hp in range(9):
                    if hp < 8:
                        for c in range(NCH):
                            scores_ew(b, hp, c, pa_tiles)
                    if hp == 0:
                        continue
                    g = hp - 1
                    av0 = avps.tile([DH + 1, 512], F32, tag="av0")
                    av1 = avps.tile([DH + 1, 512], F32, tag="av1")
                    for (c, t, st, sp) in AV_SCHED:
                        k0, kn, q0, qn = CHUNKS[c]
                        pa = pa_tiles[(g, c)]
                        lo = 128 * t - q0
                        for h, av in ((0, av0), (1, av1)):
                            nc.tensor.matmul(
                                av[0 : DH + 1, 128 * t : 128 * (t + 1)],
                                lhsT=vsb[0:kn, c, 2 * g + h, :],
                                rhs=pa[0:kn, 256 * h + lo : 256 * h + lo + 128],
                                start=st,
                                stop=sp,
                            )
                    # evac: unnormalized ctx + softmax denominator row.
                    # h0 lands rows 0:65 of ctxT (row 64 borrowed for Z, read
                    # by the gather DMA, then overwritten by h1's assembly).
                    nc.vector.tensor_copy(
                        out=ctxT[0:65, g, :], in_=av0[0 : DH + 1, :]
                    )
                    nc.gpsimd.dma_start(
                        out=zall[2 * g : 2 * g + 1, :], in_=ctxT[64:65, g, :]
                    )
                    cu = opool.tile([DH + 1, 512], BF16, tag="cu", bufs=2)
                    nc.vector.tensor_copy(out=cu, in_=av1[0 : DH + 1, :])
                    nc.gpsimd.dma_start(
                        out=zall[2 * g + 1 : 2 * g + 2, :], in_=cu[64:65, :]
                    )
                    nc.gpsimd.dma_start(
                        out=ctxT[64:128, g, :], in_=cu[0:DH, :]
                    )
                # batched 1/Z, broadcast per head-pair, in-place normalize
                zf = opool.tile([16, 512], F32, tag="zf", bufs=1)
                nc.vector.tensor_copy(out=zf, in_=zall)
                zf2 = opool.tile([16, 512], F32, tag="zf2", bufs=1)
                nc.vector.reciprocal_approx_fast(out=zf2, in_=zf)
                zr = opool.tile([16, 512], BF16, tag="zr", bufs=1)
                nc.vector.tensor_copy(out=zr, in_=zf2)
                for hp in range(8):
                    rb = opool.tile([128, 512], BF16, tag="rb", bufs=2)
                    nc.sync.dma_start(
                        out=rb,
                        in_=zr[2 * hp : 2 * hp + 2, :]
                        .unsqueeze(1)
                        .broadcast_to([2, 64, 512]),
                    )
                    nc.gpsimd.tensor_mul(ctxT[:, hp, :], ctxT[:, hp, :], rb)
                state[b] = state[b] + (ctxT,)

            def phase_f(b):
                ctxT = state[b][5]
                for t in range(KC):
                    for oc in range(2):
                        ps = mmps.tile([128, 512], F32, tag="mm")
                        for c in range(DCH):
                            nc.tensor.matmul(
                                ps,
                                lhsT=ctxT[:, c, t * 128 : (t + 1) * 128],
                                rhs=wf_sb[:, c, oc * 512 : (oc + 1) * 512],
                                start=(c == 0),
                                stop=(c == DCH - 1),
                            )
                        ob = opool.tile([128, 512], F16, tag="ob", bufs=2)
                        if use_bias_fc:
                            nc.vector.tensor_add(
                                ob, ps, bf_sb[:, oc * 512 : (oc + 1) * 512]
                            )
                        else:
                            evac(ob, ps)
                        nc.sync.dma_start(
                            out=outp[
                                b, t * 128 : (t + 1) * 128, oc * 512 : (oc + 1) * 512
                            ],
                            in_=ob,
                        )

            phase_q(0)
            phase_a(0)
            phase_q(1)
            phase_f(0)
            phase_a(1)
            phase_f(1)
    return nc


def _build_full():
    """Full-attention fallback (original kernel): data-parallel, scores
    k-major over all 4 aligned k-chunks, host-precomputed exp(dis)."""
    nc = bacc.Bacc(trn_type="TRN2")
    xb = nc.dram_tensor("xb", [BPC, L, D], F16, kind="ExternalInput")
    wqkv = nc.dram_tensor("wqkv", [D, 4 * D], F16, kind="ExternalInput")
    wfc = nc.dram_tensor("wfc", [D, D], BF16, kind="ExternalInput")
    bfc = nc.dram_tensor("bfc", [D], F32, kind="ExternalInput")
    same = nc.dram_tensor("same", [BPC, L, L], U8, kind="ExternalInput")
    gdis = nc.dram_tensor("gdis", [L, L], BF16, kind="ExternalInput")
    kbias = nc.dram_tensor("kbias", [128, BPC * KC], F32, kind="ExternalInput")
    outp = nc.dram_tensor("outp", [BPC, L, D], F32, kind="ExternalOutput")

    with tile.TileContext(nc) as tc:
        with (
            tc.tile_pool(name="w", bufs=1) as wpool,
            tc.tile_pool(name="a", bufs=1) as apool,
            tc.tile_pool(name="x", bufs=2) as xpool,
            tc.tile_pool(name="e", bufs=4) as epool,
            tc.tile_pool(name="d", bufs=2) as dpool,
            tc.tile_pool(name="o", bufs=4) as opool,
            tc.tile_pool(name="mm", bufs=2, space="PSUM") as mmps,
            tc.tile_pool(name="sc", bufs=4, space="PSUM") as scps,
            tc.tile_pool(name="av", bufs=2, space="PSUM") as avps,
        ):
            wq_sb = wpool.tile([128, DCH, 4 * D], F16)
            nc.sync.dma_start(
                out=wq_sb, in_=wqkv[:, :].rearrange("(c p) f -> p c f", p=128)
            )
            wf_sb = wpool.tile([128, DCH, D], BF16)
            nc.sync.dma_start(
                out=wf_sb, in_=wfc[:, :].rearrange("(c p) f -> p c f", p=128)
            )
            bf_sb = wpool.tile([128, D], F32)
            nc.sync.dma_start(out=bf_sb, in_=bfc[:].unsqueeze(0).broadcast_to([128, D]))
            kb_sb = wpool.tile([128, BPC * KC], F32)
            nc.sync.dma_start(out=kb_sb, in_=kbias[:, :])
            gd_sb = wpool.tile([128, KC, L], BF16)
            nc.sync.dma_start(
                out=gd_sb, in_=gdis[:, :].rearrange("(c p) q -> p c q", p=128)
            )
            ident = wpool.tile([128, 128], F16)
            make_identity(nc, ident)

            rr = [0]

            def evac(dst, src):
                if rr[0] % 2 == 0:
                    nc.vector.tensor_copy(out=dst, in_=src)
                else:
                    nc.scalar.copy(out=dst, in_=src)
                rr[0] += 1

            for b in range(BPC):
                sm = xpool.tile([128, KC, L], U8, tag="sm")
                nc.sync.dma_start(
                    out=sm, in_=same[b, :, :].rearrange("(c p) q -> p c q", p=128)
                )
                xin = xpool.tile([128, KC, D], F16, tag="xin")
                nc.sync.dma_start(
                    out=xin, in_=xb[b, :, :].rearrange("(t p) d -> p t d", p=128)
                )
                xT = xpool.tile([128, DCH, L], F16, tag="xT", bufs=1)
                for c in range(DCH):
                    for t in range(KC):
                        tp = mmps.tile([128, 128], F16, tag="mm")
                        nc.tensor.transpose(
                            tp, xin[:, t, c * 128 : (c + 1) * 128], ident
                        )
                        evac(xT[:, c, t * 128 : (t + 1) * 128], tp)

                qt = apool.tile([128, DCH, L], F16, tag="qt")
                kt1 = apool.tile([128, DCH, L], F16, tag="kt1")
                kt2 = apool.tile([128, DCH, L], F16, tag="kt2")
                vsb = apool.tile([128, KC, H, DH + 2], BF16, tag="vsb")
                ctxT = apool.tile([128, DCH, L], BF16, tag="ctxT", bufs=1)
                nc.vector.memset(vsb[:, :, :, DH : DH + 1], 1.0)

                for which, foff in ((qt, 0), (kt1, D), (kt2, 2 * D)):
                    for j in range(8):
                        ps = mmps.tile([128, L], F32, tag="mm")
                        for c in range(DCH):
                            nc.tensor.matmul(
                                ps,
                                lhsT=wq_sb[:, c, foff + j * 128 : foff + (j + 1) * 128],
                                rhs=xT[:, c, :],
                                start=(c == 0),
                                stop=(c == DCH - 1),
                            )
                        evac(which[:, j, :], ps)
                for t in range(KC):
                    for vc in range(2):
                        ps = mmps.tile([128, L], F32, tag="mm")
                        for c in range(DCH):
                            nc.tensor.matmul(
                                ps,
                                lhsT=xT[:, c, t * 128 : (t + 1) * 128],
                                rhs=wq_sb[:, c, 3 * D + vc * 512 : 3 * D + (vc + 1) * 512],
                                start=(c == 0),
                                stop=(c == DCH - 1),
                            )
                        evac(
                            vsb[:, t, vc * 8 : (vc + 1) * 8, 0:DH],
                            ps[:, :].rearrange("p (h e) -> p h e", h=8),
                        )

                Dall = dpool.tile([16, L], F32, tag="Dall")
                for hp in range(8):
                    h0, h1 = 2 * hp, 2 * hp + 1
                    av_a = avps.tile([DH + 1, L], F32, tag="av")
                    av_b = avps.tile([DH + 1, L], F32, tag="av")
                    for kc in range(KC):
                        ks = slice(kc * 128, (kc + 1) * 128)
                        s1a = scps.tile([128, L], F32, tag="sc")
                        s1b = scps.tile([128, L], F32, tag="sc")
                        s2a = scps.tile([128, L], F32, tag="sc")
                        s2b = scps.tile([128, L], F32, tag="sc")
                        nc.tensor.matmul(s1a, lhsT=kt1[0:64, hp, ks], rhs=qt[0:64, hp, :])
                        nc.tensor.matmul(s1b, lhsT=kt1[64:128, hp, ks], rhs=qt[64:128, hp, :])
                        nc.tensor.matmul(s2a, lhsT=kt2[0:64, hp, ks], rhs=qt[0:64, hp, :])
                        nc.tensor.matmul(s2b, lhsT=kt2[64:128, hp, ks], rhs=qt[64:128, hp, :])
                        nc.vector.copy_predicated(out=s2a, mask=sm[:, kc, :], data=s1a)
                        nc.vector.copy_predicated(out=s2b, mask=sm[:, kc, :], data=s1b)
                        kb = kb_sb[:, b * KC + kc : b * KC + kc + 1]
                        ea = epool.tile([128, L], BF16, tag="ea", bufs=3)
                        eb = epool.tile([128, L], BF16, tag="eb", bufs=3)
                        nc.scalar.activation(out=ea, in_=s2a, func=EXP, bias=kb, scale=1.0)
                        nc.scalar.activation(out=eb, in_=s2b, func=EXP, bias=kb, scale=1.0)
                        pa = epool.tile([128, L], BF16, tag="pa", bufs=3)
                        pb = epool.tile([128, L], BF16, tag="pb", bufs=3)
                        nc.gpsimd.tensor_mul(pa, ea, gd_sb[:, kc, :])
                        nc.gpsimd.tensor_mul(pb, eb, gd_sb[:, kc, :])
                        nc.tensor.matmul(
                            av_a, lhsT=vsb[:, kc, h0, 0 : DH + 1], rhs=pa,
                            start=(kc == 0), stop=(kc == KC - 1),
                        )
                        nc.tensor.matmul(
                            av_b, lhsT=vsb[:, kc, h1, 0 : DH + 1], rhs=pb,
                            start=(kc == 0), stop=(kc == KC - 1),
                        )
                    nc.scalar.copy(out=ctxT[0:64, hp, :], in_=av_a[0:DH, :])
                    cu = epool.tile([64, L], BF16, tag="cu")
                    nc.scalar.copy(out=cu, in_=av_b[0:DH, :])
                    nc.sync.dma_start(out=ctxT[64:128, hp, :], in_=cu)
                    da = dpool.tile([65, L], F32, tag="da")
                    db = dpool.tile([65, L], F32, tag="db")
                    nc.vector.tensor_copy(out=da[64:65, :], in_=av_a[DH : DH + 1, :])
                    nc.vector.tensor_copy(out=db[64:65, :], in_=av_b[DH : DH + 1, :])
                    nc.sync.dma_start(out=Dall[h0 : h0 + 1, :], in_=da[64:65, :])
                    nc.sync.dma_start(out=Dall[h1 : h1 + 1, :], in_=db[64:65, :])

                Rf = dpool.tile([16, L], F32, tag="Rf")
                nc.vector.reciprocal_approx_fast(out=Rf, in_=Dall)
                Rb16 = dpool.tile([16, L], BF16, tag="Rb16")
                nc.vector.tensor_copy(out=Rb16, in_=Rf)
                for hp in range(8):
                    rb = opool.tile([128, L], BF16, tag="rb")
                    nc.sync.dma_start(
                        out=rb,
                        in_=Rb16[2 * hp : 2 * hp + 2, :].unsqueeze(1).broadcast_to([2, 64, L]),
                    )
                    nc.vector.tensor_mul(ctxT[:, hp, :], ctxT[:, hp, :], rb)

                for t in range(KC):
                    for oc in range(2):
                        ps = mmps.tile([128, 512], F32, tag="mm")
                        for c in range(DCH):
                            nc.tensor.matmul(
                                ps,
                                lhsT=ctxT[:, c, t * 128 : (t + 1) * 128],
                                rhs=wf_sb[:, c, oc * 512 : (oc + 1) * 512],
                                start=(c == 0),
                                stop=(c == DCH - 1),
                            )
                        ob = opool.tile([128, 512], F32, tag="ob")
                        nc.vector.tensor_add(ob, ps, bf_sb[:, oc * 512 : (oc + 1) * 512])
                        nc.sync.dma_start(
                            out=outp[b, t * 128 : (t + 1) * 128, oc * 512 : (oc + 1) * 512],
                            in_=ob,
                        )
    return nc


_NC_CACHE = {}


def _get_nc(kind, flag=False):
    key = (kind, flag)
    if key not in _NC_CACHE:
        nc = _build_banded(flag) if kind == "banded" else _build_full()
        nc.finalize()
        _NC_CACHE[key] = nc
    return _NC_CACHE[key]


def _kernel_banded(x, mask, qmask, wq16, wf16, bfc32, shift_v):
    bf16 = ml_dtypes.bfloat16
    use_bias_fc = bool(np.any(bfc32 != 0.0))
    xT16 = np.ascontiguousarray(x.transpose(0, 2, 1).astype(np.float16))

    # same-mask chunk tiles [B, 128, NCH, 512], q-span duplicated at +256
    same_b = (qmask[:, :, None] == qmask[:, None, :]).astype(np.uint8)
    smt = np.zeros((B, 128, NCH, 512), dtype=np.uint8)
    for c, (k0, kn, q0, qn) in enumerate(CHUNKS):
        blk = same_b[:, k0 : k0 + kn, q0 : q0 + qn]  # symmetric
        smt[:, :kn, c, :qn] = blk
        smt[:, :kn, c, 256 : 256 + qn] = blk

    # Toeplitz exp(dis) tiles (bias_p cancels in softmax): variant 1 for
    # interior chunks (d = qi-kp-64), variant 0 for chunk 0 (d = qi-kp).
    kp = np.arange(128)[:, None].astype(np.float64)
    qi = np.arange(256)[None, :].astype(np.float64)
    etab = np.zeros((128, 2, 512), dtype=np.float64)
    e_int = np.exp(-shift_v * (qi - kp - 64.0) ** 2)
    e_edge = np.exp(-shift_v * (qi - kp) ** 2)
    etab[:, 1, 0:256] = e_int
    etab[:, 1, 256:512] = e_int
    etab[:, 0, 0:256] = e_edge
    etab[:, 0, 256:512] = e_edge
    etab_b = np.ascontiguousarray(etab.astype(bf16))

    keyadd = np.where(mask != 0, 0.0, NEG).astype(np.float32)  # [B, L]
    kbt = np.zeros((B, 128, NCH), dtype=np.float32)
    for c, (k0, kn, q0, qn) in enumerate(CHUNKS):
        kbt[:, :kn, c] = keyadd[:, k0 : k0 + kn]

    nc = _get_nc("banded", use_bias_fc)
    in_maps = []
    for core in range(NCORES):
        bs = slice(core * BPC, (core + 1) * BPC)
        kb_core = kbt[bs].transpose(1, 0, 2).reshape(128, BPC * NCH)
        in_maps.append(
            {
                "xTd": np.ascontiguousarray(xT16[bs]),
                "wqkv": wq16,
                "wfc": wf16,
                "bfc": bfc32,
                "smd": np.ascontiguousarray(smt[bs]),
                "etabd": etab_b,
                "kbias": np.ascontiguousarray(kb_core),
            }
        )
    res = run_bass_kernel_spmd(nc, in_maps, core_ids=list(range(NCORES)))
    kernel.last_perf = res
    out = np.concatenate([r["outp"] for r in res.results], axis=0)
    return np.ascontiguousarray(out.astype(np.float32))


def _kernel_full(x, mask, qmask, wq16, wf16, bfc32, shift_v, bias_v, ug):
    bf16 = ml_dtypes.bfloat16
    x16 = x.astype(np.float16)
    same_b = (qmask[:, :, None] == qmask[:, None, :]).astype(np.uint8)
    idx = np.arange(L, dtype=np.float64)
    if ug:
        dis = -(shift_v * (idx[:, None] - idx[None, :]) ** 2 + bias_v)
    else:
        dis = np.zeros((L, L), dtype=np.float64)
    gdis_b = np.exp(dis).astype(bf16)
    keyadd = np.where(mask != 0, 0.0, NEG).astype(np.float32)
    kb_all = keyadd.reshape(B, KC, 128).transpose(2, 0, 1)

    nc = _get_nc("full")
    in_maps = []
    for core in range(NCORES):
        bs = slice(core * BPC, (core + 1) * BPC)
        in_maps.append(
            {
                "xb": np.ascontiguousarray(x16[bs]),
                "wqkv": wq16,
                "wfc": wf16,
                "bfc": bfc32,
                "same": np.ascontiguousarray(same_b[bs]),
                "gdis": gdis_b,
                "kbias": np.ascontiguousarray(
                    kb_all[:, bs, :].reshape(128, BPC * KC)
                ),
            }
        )
    res = run_bass_kernel_spmd(nc, in_maps, core_ids=list(range(NCORES)))
    kernel.last_perf = res
    out = np.concatenate([r["outp"] for r in res.results], axis=0)
    return np.ascontiguousarray(out.astype(np.float32))


def kernel(x, mask, qmask, Wqkv, Wfc, bfc, shift, bias_p, use_Gaussian):
    bf16 = ml_dtypes.bfloat16
    x = np.asarray(x, dtype=np.float32)
    mask = np.asarray(mask)
    qmask = np.asarray(qmask)
    wq16 = np.ascontiguousarray(np.asarray(Wqkv, dtype=np.float32).astype(np.float16))
    wf16 = np.ascontiguousarray(np.asarray(Wfc, dtype=np.float32).astype(bf16))
    bfc32 = np.ascontiguousarray(np.asarray(bfc, dtype=np.float32))
    shift_v = float(np.asarray(shift, dtype=np.float64).reshape(-1)[0])
    bias_v = float(np.asarray(bias_p, dtype=np.float64).reshape(-1)[0])
    ug = bool(np.asarray(use_Gaussian).reshape(-1)[0])

    if ug and shift_v >= BAND_SHIFT_MIN:
        return _kernel_banded(x, mask, qmask, wq16, wf16, bfc32, shift_v)
    return _kernel_full(x, mask, qmask, wq16, wf16, bfc32, shift_v, bias_v, ug)
